# revision 19
# baseline (speedup 1.0000x reference)
"""Fused QK RMSNorm kernel for Trainium2 (Bass/Tile), 8-core SPMD.

Problem: q [16384, 6144], k [16384, 1024] fp32; per-row RMSNorm with
per-channel affine weight, eps=1e-6.

Sharding: rows (batch dim) split evenly across the 8 NeuronCores - each
core normalizes 2048 full rows of q and k locally. No collectives needed
(per-row stats are fully local to a core), and HBM traffic per core is
identical to the tensor-parallel split, so this is the better layout.

Per 128-row tile on a core:
  - ScalarE (ACT): Square activation with accum_out -> per-row sumsq in
    one pass over the tile.
  - ACT: sqrt(sumsq/D + eps); DVE: reciprocal -> inv_rms.
  - DVE: one fused scalar_tensor_tensor pass: out = (x * inv_rms) * w.
Everything else is DMA; the kernel is HBM-bandwidth bound
(~117 MB/core @ ~360 GB/s => ~330 us roofline).
"""

import sys

sys.path.insert(0, "/opt/trn_rl_repo")

import numpy as np

import concourse.bass as bass
import concourse.tile as tile
from concourse import mybir
from concourse.bass_utils import run_bass_kernel_spmd

B, D1, D2 = 16384, 6144, 1024
N_CORES = 8
ROWS = B // N_CORES  # rows per core
P = 128  # SBUF partitions
EPS = 1e-6

_NC_CACHE = None

# TRN2 sync-wait capacity: 1 per compute instruction, 2 for
# InstEventSemaphore (see bacc.generate_event_semaphores docstring).
MAX_KEEP = 1
EV_CHUNK = 2


def _legalize_waits(nc):
    """Split excess sync waits onto InstEventSemaphore instructions
    inserted right before the overloaded instruction (same engine, wait-
    only). Mirrors bacc's generate_event_semaphores, which doesn't run in
    the BIR->walrus path."""
    n_split = 0
    for f in nc.m.functions:
        for blk in f.blocks:
            insts = blk.instructions  # live list
            i = 0
            while i < len(insts):
                inst = insts[i]
                si = inst.sync_info
                waits = list(si.on_wait) if si and si.on_wait else []
                cap = EV_CHUNK if isinstance(
                    inst, mybir.InstEventSemaphore) else MAX_KEEP
                if len(waits) <= cap:
                    i += 1
                    continue
                keep = waits[-cap:]
                excess = waits[:-cap]
                pos = i
                while excess:
                    chunk, excess = excess[:EV_CHUNK], excess[EV_CHUNK:]
                    ev = mybir.InstEventSemaphore(
                        name=f"waitsplit_{n_split}",
                        engine=inst.engine,
                        ins=[],
                        outs=[],
                        sync_info=mybir.SyncInfo(on_wait=chunk, on_update=[]),
                    )
                    n_split += 1
                    insts.insert(pos, ev)
                    pos += 1
                inst.sync_info = mybir.SyncInfo(
                    on_wait=keep, on_update=list(si.on_update)
                )
                i = pos + 1
    return n_split


def _build_nc():
    f32 = mybir.dt.float32
    nc = bass.Bass()

    q = nc.declare_dram_parameter("q", [ROWS, D1], f32, isOutput=False)
    k = nc.declare_dram_parameter("k", [ROWS, D2], f32, isOutput=False)
    qw = nc.declare_dram_parameter("q_weight", [D1], f32, isOutput=False)
    kw = nc.declare_dram_parameter("k_weight", [D2], f32, isOutput=False)
    out_q = nc.declare_dram_parameter("out_q", [ROWS, D1], f32, isOutput=True)
    out_k = nc.declare_dram_parameter("out_k", [ROWS, D2], f32, isOutput=True)

    ntiles = ROWS // P
    mult = mybir.AluOpType.mult

    with (
        tile.TileContext(nc) as tc,
        tc.tile_pool(name="qpool", bufs=2) as qpool,
        tc.tile_pool(name="kpool", bufs=2) as kpool,
        tc.tile_pool(name="opool", bufs=2) as opool,
        tc.tile_pool(name="scratch", bufs=1) as scratch,
        tc.tile_pool(name="singles", bufs=1) as singles,
        tc.tile_pool(name="stats", bufs=8) as stats,
        tc.tile_pool(name="psum", bufs=2, space="PSUM") as psum_pool,
    ):
        # Load each weight vector into partition 0 once (24 KB / 4 KB HBM
        # reads), then broadcast on-chip via GPSIMD - avoids 3.7 MB of
        # broadcast HBM reads on the bandwidth-bound path.
        qw_row = singles.tile([1, D1], f32)
        nc.sync.dma_start(out=qw_row, in_=qw[:].unsqueeze(0))
        kw_row = singles.tile([1, D2], f32)
        nc.sync.dma_start(out=kw_row, in_=kw[:].unsqueeze(0))

        # Broadcast partition 0 -> all 128 partitions via PE outer
        # product (ones[1,128].T @ w[1,512] per PSUM bank), with the
        # PSUM->SBUF copies on DVE so the DVE applies inherit the weight
        # dependency in-order (no extra sync waits).
        ones = singles.tile([1, P], f32)
        nc.vector.memset(ones, 1.0)
        qw_b = singles.tile([P, D1], f32)
        kw_b = singles.tile([P, D2], f32)
        for w_row, w_b, d in ((qw_row, qw_b, D1), (kw_row, kw_b, D2)):
            for c in range(0, d, 512):
                pt = psum_pool.tile([P, 512], f32)
                nc.tensor.matmul(
                    pt, ones, w_row[:, c:c + 512], start=True, stop=True
                )
                nc.vector.tensor_copy(out=w_b[:, c:c + 512], in_=pt)

        def rmsnorm_tile(x_dram, o_dram, w_b, d, i, pool, sq_tag):
            r0, r1 = i * P, (i + 1) * P
            x = pool.tile([P, d], f32)
            nc.sync.dma_start(out=x, in_=x_dram[r0:r1, :])

            # sumsq[p] = sum_f x[p,f]^2 in one ACT pass; the full-size
            # Square output is a write-only scratch.
            sq = scratch.tile([P, d], f32, tag=sq_tag)
            ss = stats.tile([P, 1], f32, tag="ss")
            nc.scalar.activation(
                out=sq, in_=x, func=mybir.ActivationFunctionType.Square,
                accum_out=ss,
            )
            # t = sumsq/d + eps; rms = sqrt(t); inv_rms = 1/rms
            t = stats.tile([P, 1], f32, tag="t")
            nc.vector.tensor_scalar(
                out=t, in0=ss, scalar1=1.0 / d, scalar2=EPS,
                op0=mult, op1=mybir.AluOpType.add,
            )
            rms = stats.tile([P, 1], f32, tag="rms")
            nc.scalar.sqrt(rms, t)
            inv = stats.tile([P, 1], f32, tag="inv")
            nc.vector.reciprocal(out=inv, in_=rms)

            # out = (x * inv_rms) * w in one DVE pass. Out-of-place, with
            # a tiny memset pre-touch of the output slot: the memset
            # carries the WAR-on-previous-store sync wait so the apply
            # stays within walrus's 2-wait limit.
            o = opool.tile([P, d], f32, tag="o_" + sq_tag)
            nc.vector.memset(o[:, 0:1], 0.0)
            nc.vector.scalar_tensor_tensor(
                out=o, in0=x, scalar=inv, in1=w_b, op0=mult, op1=mult,
            )
            # Stores go out on the ACT HWDGE ring (loads use the SP ring)
            # so the two descriptor streams interleave across SDMA engines.
            nc.scalar.dma_start(out=o_dram[r0:r1, :], in_=o)

        for i in range(ntiles):
            rmsnorm_tile(q, out_q, qw_b, D1, i, qpool, "sq_q")
            rmsnorm_tile(k, out_k, kw_b, D2, i, kpool, "sq_k")

    _legalize_waits(nc)
    return nc


def _get_nc():
    global _NC_CACHE
    if _NC_CACHE is None:
        _NC_CACHE = _build_nc()
    return _NC_CACHE


def _shard(inputs):
    q = np.ascontiguousarray(np.asarray(inputs["q"], dtype=np.float32))
    k = np.ascontiguousarray(np.asarray(inputs["k"], dtype=np.float32))
    qw = np.ascontiguousarray(np.asarray(inputs["q_weight"], dtype=np.float32))
    kw = np.ascontiguousarray(np.asarray(inputs["k_weight"], dtype=np.float32))
    in_maps = []
    for c in range(N_CORES):
        r0, r1 = c * ROWS, (c + 1) * ROWS
        in_maps.append(
            {"q": q[r0:r1], "k": k[r0:r1], "q_weight": qw, "k_weight": kw}
        )
    return in_maps


def _run(in_maps, trace=False, **kwargs):
    return run_bass_kernel_spmd(
        _get_nc(), in_maps, core_ids=list(range(N_CORES)), trace=trace, **kwargs
    )


def kernel(**inputs):
    res = _run(_shard(inputs))
    out_q = np.concatenate([r["out_q"] for r in res.results], axis=0)
    out_k = np.concatenate([r["out_k"] for r in res.results], axis=0)
    return out_q, out_k


if __name__ == "__main__":
    rng = np.random.default_rng(0)
    inputs = {
        "q": rng.standard_normal((B, D1), dtype=np.float32),
        "k": rng.standard_normal((B, D2), dtype=np.float32),
        "q_weight": rng.random(D1, dtype=np.float32),
        "k_weight": rng.random(D2, dtype=np.float32),
    }
    oq, ok = kernel(**inputs)
    print(oq.shape, ok.shape, oq.dtype, ok.dtype)


# revision 20
# speedup vs baseline: 1.2831x; 1.2831x over previous
"""Fused QK RMSNorm kernel for Trainium2 (Bass/Tile), 8-core SPMD.

Problem: q [16384, 6144], k [16384, 1024] fp32; per-row RMSNorm with
per-channel affine weight, eps=1e-6.

Sharding: rows (batch dim) split evenly across the 8 NeuronCores - each
core normalizes 2048 full rows of q and k locally. No collectives needed
(per-row stats are fully local to a core), and HBM traffic per core is
identical to the tensor-parallel split, so this is the better layout.

Per 128-row tile on a core:
  - ScalarE (ACT): Square activation with accum_out -> per-row sumsq in
    one pass over the tile.
  - ACT: sqrt(sumsq/D + eps); DVE: reciprocal -> inv_rms.
  - DVE: one fused scalar_tensor_tensor pass: out = (x * inv_rms) * w.
Everything else is DMA; the kernel is HBM-bandwidth bound
(~117 MB/core @ ~360 GB/s => ~330 us roofline).
"""

import sys

sys.path.insert(0, "/opt/trn_rl_repo")

import numpy as np

import concourse.bass as bass
import concourse.tile as tile
from concourse import mybir
from concourse.bass_utils import run_bass_kernel_spmd

B, D1, D2 = 16384, 6144, 1024
N_CORES = 8
ROWS = B // N_CORES  # rows per core
P = 128  # SBUF partitions
EPS = 1e-6

_NC_CACHE = None

# TRN2 sync-wait capacity: 1 per compute instruction, 2 for
# InstEventSemaphore (see bacc.generate_event_semaphores docstring).
MAX_KEEP = 1
EV_CHUNK = 2


def _legalize_waits(nc):
    """Split excess sync waits onto InstEventSemaphore instructions
    inserted right before the overloaded instruction (same engine, wait-
    only). Mirrors bacc's generate_event_semaphores, which doesn't run in
    the BIR->walrus path."""
    n_split = 0
    for f in nc.m.functions:
        for blk in f.blocks:
            insts = blk.instructions  # live list
            i = 0
            while i < len(insts):
                inst = insts[i]
                si = inst.sync_info
                waits = list(si.on_wait) if si and si.on_wait else []
                cap = EV_CHUNK if isinstance(
                    inst, mybir.InstEventSemaphore) else MAX_KEEP
                if len(waits) <= cap:
                    i += 1
                    continue
                keep = waits[-cap:]
                excess = waits[:-cap]
                pos = i
                while excess:
                    chunk, excess = excess[:EV_CHUNK], excess[EV_CHUNK:]
                    ev = mybir.InstEventSemaphore(
                        name=f"waitsplit_{n_split}",
                        engine=inst.engine,
                        ins=[],
                        outs=[],
                        sync_info=mybir.SyncInfo(on_wait=chunk, on_update=[]),
                    )
                    n_split += 1
                    insts.insert(pos, ev)
                    pos += 1
                inst.sync_info = mybir.SyncInfo(
                    on_wait=keep, on_update=list(si.on_update)
                )
                i = pos + 1
    return n_split


def _build_nc():
    f32 = mybir.dt.float32
    nc = bass.Bass()

    q = nc.declare_dram_parameter("q", [ROWS, D1], f32, isOutput=False)
    k = nc.declare_dram_parameter("k", [ROWS, D2], f32, isOutput=False)
    qw = nc.declare_dram_parameter("q_weight", [D1], f32, isOutput=False)
    kw = nc.declare_dram_parameter("k_weight", [D2], f32, isOutput=False)
    out_q = nc.declare_dram_parameter("out_q", [ROWS, D1], f32, isOutput=True)
    out_k = nc.declare_dram_parameter("out_k", [ROWS, D2], f32, isOutput=True)

    ntiles = ROWS // P
    mult = mybir.AluOpType.mult

    with (
        tile.TileContext(nc) as tc,
        tc.tile_pool(name="qpool", bufs=2) as qpool,
        tc.tile_pool(name="kpool", bufs=2) as kpool,
        tc.tile_pool(name="opool", bufs=2) as opool,
        tc.tile_pool(name="scratch", bufs=1) as scratch,
        tc.tile_pool(name="singles", bufs=1) as singles,
        tc.tile_pool(name="stats", bufs=8) as stats,
        tc.tile_pool(name="psum", bufs=2, space="PSUM") as psum_pool,
    ):
        # Load each weight vector into partition 0 once (24 KB / 4 KB HBM
        # reads), then broadcast on-chip via GPSIMD - avoids 3.7 MB of
        # broadcast HBM reads on the bandwidth-bound path.
        qw_row = singles.tile([1, D1], f32)
        nc.sync.dma_start(out=qw_row, in_=qw[:].unsqueeze(0))
        kw_row = singles.tile([1, D2], f32)
        nc.sync.dma_start(out=kw_row, in_=kw[:].unsqueeze(0))

        # Broadcast partition 0 -> all 128 partitions via PE outer
        # product (ones[1,128].T @ w[1,512] per PSUM bank), with the
        # PSUM->SBUF copies on DVE so the DVE applies inherit the weight
        # dependency in-order (no extra sync waits).
        ones = singles.tile([1, P], f32)
        nc.vector.memset(ones, 1.0)
        qw_b = singles.tile([P, D1], f32)
        kw_b = singles.tile([P, D2], f32)
        for w_row, w_b, d in ((qw_row, qw_b, D1), (kw_row, kw_b, D2)):
            for c in range(0, d, 512):
                pt = psum_pool.tile([P, 512], f32)
                nc.tensor.matmul(
                    pt, ones, w_row[:, c:c + 512], start=True, stop=True
                )
                nc.vector.tensor_copy(out=w_b[:, c:c + 512], in_=pt)

        def rmsnorm_tile(x_dram, o_dram, w_b, d, i, pool, sq_tag):
            r0, r1 = i * P, (i + 1) * P
            x = pool.tile([P, d], f32)
            nc.sync.dma_start(out=x, in_=x_dram[r0:r1, :])

            # sumsq[p] = sum_f x[p,f]^2 in one ACT pass; the full-size
            # Square output is a write-only scratch.
            sq = scratch.tile([P, d], f32, tag=sq_tag)
            ss = stats.tile([P, 1], f32, tag="ss")
            nc.scalar.activation(
                out=sq, in_=x, func=mybir.ActivationFunctionType.Square,
                accum_out=ss,
            )
            # t = sumsq/d + eps; rms = sqrt(t); inv_rms = 1/rms
            t = stats.tile([P, 1], f32, tag="t")
            nc.vector.tensor_scalar(
                out=t, in0=ss, scalar1=1.0 / d, scalar2=EPS,
                op0=mult, op1=mybir.AluOpType.add,
            )
            rms = stats.tile([P, 1], f32, tag="rms")
            nc.scalar.sqrt(rms, t)
            inv = stats.tile([P, 1], f32, tag="inv")
            nc.vector.reciprocal(out=inv, in_=rms)

            # out = (x * inv_rms) * w in one DVE pass. Out-of-place, with
            # a tiny memset pre-touch of the output slot: the memset
            # carries the WAR-on-previous-store sync wait so the apply
            # stays within walrus's 2-wait limit.
            o = opool.tile([P, d], f32, tag="o_" + sq_tag)
            nc.vector.memset(o[:, 0:1], 0.0)
            nc.vector.scalar_tensor_tensor(
                out=o, in0=x, scalar=inv, in1=w_b, op0=mult, op1=mult,
            )
            nc.sync.dma_start(out=o_dram[r0:r1, :], in_=o)

        for i in range(ntiles):
            rmsnorm_tile(q, out_q, qw_b, D1, i, qpool, "sq_q")
            rmsnorm_tile(k, out_k, kw_b, D2, i, kpool, "sq_k")

    _legalize_waits(nc)
    return nc


def _get_nc():
    global _NC_CACHE
    if _NC_CACHE is None:
        _NC_CACHE = _build_nc()
    return _NC_CACHE


def _shard(inputs):
    q = np.ascontiguousarray(np.asarray(inputs["q"], dtype=np.float32))
    k = np.ascontiguousarray(np.asarray(inputs["k"], dtype=np.float32))
    qw = np.ascontiguousarray(np.asarray(inputs["q_weight"], dtype=np.float32))
    kw = np.ascontiguousarray(np.asarray(inputs["k_weight"], dtype=np.float32))
    in_maps = []
    for c in range(N_CORES):
        r0, r1 = c * ROWS, (c + 1) * ROWS
        in_maps.append(
            {"q": q[r0:r1], "k": k[r0:r1], "q_weight": qw, "k_weight": kw}
        )
    return in_maps


def _run(in_maps, trace=False, **kwargs):
    return run_bass_kernel_spmd(
        _get_nc(), in_maps, core_ids=list(range(N_CORES)), trace=trace, **kwargs
    )


def kernel(**inputs):
    res = _run(_shard(inputs))
    out_q = np.concatenate([r["out_q"] for r in res.results], axis=0)
    out_k = np.concatenate([r["out_k"] for r in res.results], axis=0)
    return out_q, out_k


if __name__ == "__main__":
    rng = np.random.default_rng(0)
    inputs = {
        "q": rng.standard_normal((B, D1), dtype=np.float32),
        "k": rng.standard_normal((B, D2), dtype=np.float32),
        "q_weight": rng.random(D1, dtype=np.float32),
        "k_weight": rng.random(D2, dtype=np.float32),
    }
    oq, ok = kernel(**inputs)
    print(oq.shape, ok.shape, oq.dtype, ok.dtype)
